# revision 32
# baseline (speedup 1.0000x reference)
"""Trainium2 Bass kernel for BlockAxialDown (maxpool + axial attention + 1x1 conv + batchnorm).

Contract: kernel(**inputs) takes FULL unsharded inputs, returns FULL output.
Sharding: data-parallel over batch B=8 across 8 NeuronCores (1 image/core);
BatchNorm batch stats combined with a tiny (128,4) AllReduce; weights replicated.

v1 redesign vs the 524us baseline (PE 96% busy but at 1.3-1.8 ns/col):
- 3-stage software pipeline over a flat list of 64 groups (32 W + 32 H):
  stage_a(g)=projections, stage_b(g-1)=dots+exp, stage_c(g-2)=sums/AV/out.
  PE sees a dense stream with one-iteration slack on every cross-engine
  dependency, so LDWEIGHTS hides and the p-state ramp stays warm.
- Per-group softmax normalization: og = oT / sums via one DVE divide
  (fallback: reciprocal_approx_fast+mul). Kills the deferred-ln scheme, its
  32KB SBUF spill, two ACT passes and all ACT table switches (exp set only).
- Engine rebalance: q-spill on ACT, k/v-spills on DVE (k on ACT during H),
  W-acc on ACT via Identity+bias, H-acc on DVE, maxpool + post-attn relu on
  the previously idle GPSIMD, exp batched per head.
- Tail: affine split across ACT/DVE/GPSIMD, output DMAs issued per chunk.
- rstd via exp(-0.5*ln(v+eps)) instead of sqrt+reciprocal: stays in the one
  ACT table set.
"""

import sys

import numpy as np

for _p in ("/opt/trn_rl_repo", "/root/.axon_site/_ro/trn_rl_repo"):
    if _p not in sys.path:
        sys.path.append(_p)

B, C, H, W = 8, 128, 256, 256
H2, W2 = 128, 128
E = 2 * C
NPOS = H2 * W2
NCORES = 8
BN_EPS = 1e-5
DH = C // 2
SCALE = DH ** -0.5

USE_DIVIDE = False  # AluOpType.divide returns 0.0 on TRN2 DVE (unimplemented)

_CACHE = {}


def _build_program():
    import concourse.tile as tile
    from concourse import bacc, library_config, mybir
    from concourse.alu_op_type import AluOpType
    from contextlib import ExitStack

    F32 = mybir.dt.float32
    BF16 = mybir.dt.bfloat16
    AF = mybir.ActivationFunctionType
    P = 128

    nc = bacc.Bacc("TRN2", target_bir_lowering=False, debug=False, num_devices=NCORES)

    # ---- DRAM I/O ----
    x_d = nc.dram_tensor("x", [C, H, W], BF16, kind="ExternalInput").ap()
    wq_w_d = nc.dram_tensor("wq_w", [C, C], BF16, kind="ExternalInput").ap()
    wk_w_d = nc.dram_tensor("wk_w", [C, C], BF16, kind="ExternalInput").ap()
    wq_h_d = nc.dram_tensor("wq_h", [C, C], BF16, kind="ExternalInput").ap()
    wk_h_d = nc.dram_tensor("wk_h", [C, C], BF16, kind="ExternalInput").ap()
    wv_w_d = nc.dram_tensor("wv_w", [C, C], BF16, kind="ExternalInput").ap()
    wo_w_d = nc.dram_tensor("wo_w", [C, C], BF16, kind="ExternalInput").ap()
    wv_h_d = nc.dram_tensor("wv_h", [C, C], BF16, kind="ExternalInput").ap()
    wo_h_d = nc.dram_tensor("wo_h", [C, C], BF16, kind="ExternalInput").ap()
    bsum_d = nc.dram_tensor("bsum", [C, 1], F32, kind="ExternalInput").ap()
    convA_d = nc.dram_tensor("convA", [C, E], BF16, kind="ExternalInput").ap()
    convX_d = nc.dram_tensor("convX", [C, E], BF16, kind="ExternalInput").ap()
    gamma2_d = nc.dram_tensor("gamma2", [C, 2], F32, kind="ExternalInput").ap()
    beta2_d = nc.dram_tensor("beta2", [C, 2], F32, kind="ExternalInput").ap()
    out_d = nc.dram_tensor("out", [E, H2, W2], F32, kind="ExternalOutput").ap()
    stats_in_d = nc.dram_tensor("stats_in", [P, 4], F32).ap()
    stats_out_d = nc.dram_tensor("stats_out", [P, 4], F32, addr_space="Shared").ap()

    with tile.TileContext(nc) as tc, ExitStack() as ctx:
        const = ctx.enter_context(tc.tile_pool(name="const", bufs=1))
        cube = ctx.enter_context(tc.tile_pool(name="cube", bufs=1))
        stage = ctx.enter_context(tc.tile_pool(name="stage", bufs=3))
        work = ctx.enter_context(tc.tile_pool(name="work", bufs=2))
        stats = ctx.enter_context(tc.tile_pool(name="stats", bufs=1))
        psum = ctx.enter_context(tc.tile_pool(name="psum", bufs=1, space="PSUM"))

        # ---- weights via ACT HWDGE (small, land in a few us); inputs via SP
        # so neither queue-drain nor issue-serialization gates chunk 0 ----
        def cload(name, ap_d, shape, dt):
            t = const.tile(shape, dt, name=name)
            nc.scalar.dma_start(out=t[:], in_=ap_d)
            return t

        m_w = [cload("wq_w_t", wq_w_d, [C, C], BF16),
               cload("wk_w_t", wk_w_d, [C, C], BF16)]
        m_h = [cload("wq_h_t", wq_h_d, [C, C], BF16),
               cload("wk_h_t", wk_h_d, [C, C], BF16)]
        wv_w = cload("wv_w_t", wv_w_d, [C, C], BF16)
        wo_w = cload("wo_w_t", wo_w_d, [C, C], BF16)
        wv_h = cload("wv_h_t", wv_h_d, [C, C], BF16)
        wo_h = cload("wo_h_t", wo_h_d, [C, C], BF16)
        bsum = cload("bsum_t", bsum_d, [C, 1], F32)
        convA = cload("convA_t", convA_d, [C, E], BF16)
        convX = cload("convX_t", convX_d, [C, E], BF16)
        gamma2 = cload("gamma2_t", gamma2_d, [C, 2], F32)
        beta2 = cload("beta2_t", beta2_d, [C, 2], F32)
        ones64 = const.tile([P, 64], BF16, name="ones64")
        nc.vector.memset(ones64[:], 1.0)


        xp = cube.tile([P, H2, W2], BF16)   # pooled input, channels on partitions
        acc = cube.tile([P, H2, W2], BF16)  # attention output accumulator
        S = cube.tile([P, 2 * NPOS], BF16)  # scratch: xpT -> y2
        xp_f = xp[:].rearrange("c h w -> c (h w)")
        acc_f = acc[:].rearrange("c h w -> c (h w)")
        xpT_v = S[:, 0:NPOS].rearrange("c (w h) -> c w h", h=H2)   # (c, 128, 128)
        y2_v = S[:].rearrange("c (n e i) -> c n e i", e=2, i=512)  # conv output store

        # ---- input DMA: all issued up front on SP; early chunks split into
        # quarters across queues so chunk 0 lands in ~7us ----
        xins = []
        xv = x_d.rearrange("c (n h) w -> c n h w", h=8)
        for i in range(H // 8):
            xin = stage.tile([P, 8, W], BF16, tag="xin")
            if i < 4:
                for q in range(4):
                    nc.sync.dma_start(out=xin[:, 2 * q:2 * q + 2, :],
                                      in_=xv[:, i, 2 * q:2 * q + 2, :])
            elif i < 16:
                for q in range(2):
                    nc.sync.dma_start(out=xin[:, 4 * q:4 * q + 4, :],
                                      in_=xv[:, i, 4 * q:4 * q + 4, :])
            else:
                nc.sync.dma_start(out=xin[:], in_=xv[:, i])
            xins.append(xin)

        # ---- ACT table prewarm (exp set; loads during input DMA wait) ----
        warm = stats.tile([P, 8], F32)
        nc.vector.memset(warm[:], 1.0)
        nc.scalar.activation(warm[:, 0:1], warm[:, 4:5], AF.Exp)

        # ---- PE warmup: dummy matmuls so the p-state ramp starts before the
        # first input chunk lands ----
        wups = psum.tile([P, 512], F32, tag="yg", name="wups")
        for r in range(4):
            nc.tensor.matmul(wups[:, 128 * r:128 * r + 128], lhsT=m_w[0][:],
                             rhs=m_w[1][:], start=True, stop=True)
        nc.vector.tensor_copy(warm[:, 4:8], wups[:, 0:4])

        def maxpool_chunk(i):
            # 2x2 maxpool of raw chunk i (8 rows) -> xp rows 4i..4i+4 on DVE
            # (GPSIMD cannot run TensorTensor in this compile path); then the
            # transposed copy of those 4 rows into xpT columns on GPSIMD
            # (incremental build, no W->H boundary stall)
            xin = xins[i]
            xw = work.tile([P, 4, W], BF16, tag="xw", bufs=2)
            xin_v = xin[:].rearrange("c (r two) w -> c r two w", two=2)
            nc.vector.tensor_max(xw[:], xin_v[:, :, 0, :], xin_v[:, :, 1, :])
            xw4 = xw[:].rearrange("c r (w two) -> c r w two", two=2)
            nc.vector.tensor_max(xp[:, 4 * i:4 * i + 4, :],
                                 xw4[:, :, :, 0], xw4[:, :, :, 1])
            src = xp[:, 4 * i:4 * i + 4, :].rearrange("c h w -> c w h")
            nc.gpsimd.tensor_copy(xpT_v[:, :, 4 * i:4 * i + 4], src)

        # ---------------- pipelined axial attention ----------------
        # group descriptor: (src_v, m01, wv, wo, is_w, g)
        groups = []
        for g in range(32):
            groups.append(("w", g))
        for g in range(32):
            groups.append(("h", g))

        def src_of(d, g):
            if d == "w":
                return xp[:, 4 * g:4 * g + 4, :]
            return xpT_v[:, 4 * g:4 * g + 4, :]

        def stage_a_mm(d, g):
            # projection matmuls: q,k -> mkps [c,1024]; v per slice -> vps
            xg = src_of(d, g)
            xg_f = xg.rearrange("c s i -> c (s i)")
            m01 = m_w if d == "w" else m_h
            wv = wv_w if d == "w" else wv_h
            mkps = psum.tile([P, 1024], F32, tag="mkps", name="mkps")
            for h in range(2):
                nc.tensor.matmul(mkps[:, 512 * h:512 * h + 512], lhsT=m01[h][:],
                                 rhs=xg_f, start=True, stop=True)
            vps = psum.tile([P, 512], F32, tag="vps", name="vps")
            for s in range(4):
                nc.tensor.matmul(vps[:, 128 * s:128 * s + 128], lhsT=xg[:, s, :],
                                 rhs=wv[:], start=True, stop=True)
            return mkps, vps

        def stage_a_spill(d, g, mkps, vps):
            mk = work.tile([P, 1024], BF16, tag="mk", bufs=2)
            # q+k spills on ACT (one batched copy); v-spill on DVE
            nc.scalar.copy(mk[:], mkps[:])
            vs = work.tile([P, 512], BF16, tag="vs", bufs=3)
            nc.vector.tensor_copy(vs[:], vps[:])
            return mk, vs

        def stage_b(d, g, mk):
            # dots (transposed) into one 2-bank psum; one batched exp.
            # Heads interleaved: consecutive matmuls use disjoint PE row
            # groups (head h lives on partitions 64h..64h+64), so LDWEIGHTS
            # of one head pulls ahead of the other head's in-flight matmul.
            dT = psum.tile([P, 1024], F32, tag="dT", name="dT")
            for s in range(4):
                for h in range(2):
                    hp = slice(64 * h, 64 * h + 64)
                    qs = mk[hp, 128 * s:128 * s + 128]
                    ks = mk[hp, 512 + 128 * s:512 + 128 * s + 128]
                    nc.tensor.matmul(dT[:, 512 * h + 128 * s:512 * h + 128 * s + 128],
                                     lhsT=ks, rhs=qs, start=True, stop=True)
            e = work.tile([P, 1024], BF16, tag="e", bufs=2)
            nc.scalar.activation(e[:], dT[:], AF.Exp, scale=SCALE)
            return e

        def stage_c_main(d, g, vs, e):
            # softmax sums broadcast via ones-matmul; AV; normalize; out-proj
            wo = wo_w if d == "w" else wo_h
            bc = psum.tile([P, 512], F32, tag="bc", name="bc")
            for h in range(2):
                nc.tensor.matmul(bc[64 * h:64 * h + 64, :], lhsT=ones64[:],
                                 rhs=e[:, 512 * h:512 * h + 512],
                                 start=True, stop=True, tile_position=(0, 64 * h))
            # reciprocal early on DVE (bc ready from prior iteration's end)
            rcp = work.tile([P, 512], F32, tag="rcp", bufs=2)
            nc.vector.reciprocal_approx_fast(out=rcp[:], in_=bc[:])
            oT = psum.tile([P, 512], F32, tag="oT", name="oT")
            for s in range(4):
                for h in range(2):
                    nc.tensor.matmul(
                        oT[64 * h:64 * h + 64, 128 * s:128 * s + 128],
                        lhsT=vs[:, 128 * s + 64 * h:128 * s + 64 * h + 64],
                        rhs=e[:, 512 * h + 128 * s:512 * h + 128 * s + 128],
                        start=True, stop=True, tile_position=(0, 64 * h))
            og = work.tile([P, 512], BF16, tag="og", bufs=2)
            nc.vector.tensor_mul(og[:], oT[:], rcp[:])
            yg = psum.tile([P, 512], F32, tag="yg", name="yg")
            nc.tensor.matmul(yg[:], lhsT=wo[:], rhs=og[:], start=True, stop=True)
            return yg

        def stage_c_acc(d, g, yg):
            if d == "w":
                # acc = yg + (bout_h + bout_w): ACT identity with bias
                nc.scalar.activation(acc_f[:, 512 * g:512 * (g + 1)], yg[:],
                                     AF.Identity, bias=bsum[:, 0:1])
            else:
                # accumulate transposed: acc[:, i, 4g+s] += yg[:, (s, i)]
                yg_r = yg[:].rearrange("c (s i) -> c i s", s=4)
                acc_sl = acc[:, :, 4 * g:4 * g + 4]
                nc.vector.tensor_add(acc_sl, acc_sl, yg_r)
                # relu(a_h + a_w) for these 4 columns on ACT (same table set)
                nc.scalar.activation(acc_sl, acc_sl, AF.Relu)

        NG = len(groups)
        live = {}
        for t in range(NG + 2):
            # prefetch maxpool one chunk ahead of stage_a use
            if t < 32:
                if t == 0:
                    maxpool_chunk(0)
                if t + 1 < 32:
                    maxpool_chunk(t + 1)
            if 1 <= t < NG + 1:
                d, g, mkps, vps, mk, vs = live[t - 1]
                e = stage_b(d, g, mk)
                live[t - 1] = (d, g, mk, vs, e)
            if t < NG:
                d, g = groups[t]
                mkps, vps = stage_a_mm(d, g)
            if t >= 2:
                dd, gg, mk2, vs2, e2 = live.pop(t - 2)
                yg = stage_c_main(dd, gg, vs2, e2)
            if t < NG:
                d, g = groups[t]
                mk, vs = stage_a_spill(d, g, mkps, vps)
                live[t] = (d, g, mkps, vps, mk, vs)
            if t >= 2:
                stage_c_acc(dd, gg, yg)

        # ---------------- conv 1x1 + relu + bn stats ----------------
        bnb = [stats.tile([P, 32, 6], F32, name=f"bnb{i}") for i in range(2)]
        for p in range(NPOS // 512):
            pos = slice(512 * p, 512 * (p + 1))
            for eh in range(2):
                yps = psum.tile([P, 512], F32, tag=("oT" if eh else "bc"),
                                name="yps")
                ce = slice(128 * eh, 128 * eh + 128)
                nc.tensor.matmul(yps[:], lhsT=convA[:, ce], rhs=acc_f[:, pos],
                                 start=True, stop=False)
                nc.tensor.matmul(yps[:], lhsT=convX[:, ce], rhs=xp_f[:, pos],
                                 start=False, stop=True)
                nc.scalar.activation(y2_v[:, p, eh], yps[:], AF.Relu)
                nc.vector.bn_stats(bnb[eh][:, p, :], y2_v[:, p, eh])

        # ---------------- stats aggregate + AllReduce + coefficients ----------------
        mv = stats.tile([P, 2, 2], F32)
        for eh in range(2):
            nc.vector.bn_aggr(mv[:, eh, :], bnb[eh][:])
        cc_in = stats.tile([P, 4], F32)
        for eh in range(2):
            # [mean, E[y^2]] per half; E[y^2] = var + mean^2
            nc.vector.tensor_copy(cc_in[:, 2 * eh:2 * eh + 1], mv[:, eh, 0:1])
            nc.vector.scalar_tensor_tensor(
                cc_in[:, 2 * eh + 1:2 * eh + 2],
                in0=mv[:, eh, 0:1], scalar=mv[:, eh, 0:1], in1=mv[:, eh, 1:2],
                op0=AluOpType.mult, op1=AluOpType.add)
        nc.sync.dma_start(out=stats_in_d, in_=cc_in[:])
        nc.gpsimd.collective_compute(
            "AllReduce", AluOpType.add,
            replica_groups=[list(range(NCORES))],
            ins=[stats_in_d], outs=[stats_out_d])
        gst = stats.tile([P, 4], F32)
        nc.sync.dma_start(out=gst[:], in_=stats_out_d)

        t0 = stats.tile([P, 4], F32)
        nc.vector.tensor_scalar_mul(t0[:], gst[:], 1.0 / NCORES)
        t0v = t0[:].rearrange("c (e two) -> c e two", two=2)
        m2 = stats.tile([P, 2], F32)
        veps = stats.tile([P, 2], F32)
        means = stats.tile([P, 2], F32)
        for eh in range(2):
            nc.vector.tensor_copy(means[:, eh:eh + 1], t0v[:, eh, 0:1])
            nc.vector.tensor_mul(m2[:, eh:eh + 1], t0v[:, eh, 0:1], t0v[:, eh, 0:1])
            nc.vector.scalar_tensor_tensor(
                veps[:, eh:eh + 1],
                in0=t0v[:, eh, 1:2], scalar=BN_EPS, in1=m2[:, eh:eh + 1],
                op0=AluOpType.add, op1=AluOpType.subtract)
        # rstd = exp(-0.5 * ln(var + eps)): stays in the exp/ln table set
        lv = stats.tile([P, 2], F32)
        nc.scalar.activation(lv[:], veps[:], AF.Ln)
        rstd = stats.tile([P, 2], F32)
        nc.scalar.activation(rstd[:], lv[:], AF.Exp, scale=-0.5)
        scl = stats.tile([P, 2], F32)
        nc.vector.tensor_mul(scl[:], gamma2[:], rstd[:])
        msc = stats.tile([P, 2], F32)
        nc.vector.tensor_mul(msc[:], means[:], scl[:])
        shift = stats.tile([P, 2], F32)
        nc.vector.tensor_sub(shift[:], beta2[:], msc[:])

        # ---------------- affine + output DMA ----------------
        # Affine on ACT/DVE into wide f32 staging tiles; one fat 1MB DMA per
        # 2048 positions (16 total, queue-parallel) instead of 64 serialized
        # SP issues of 256KB.
        out_r = out_d.rearrange("(two c) h w -> two c (h w)", two=2)
        unit = 0
        for eh in range(2):
            for blk in range(16):
                yo2 = work.tile([P, 1024], F32, tag="yo4", bufs=8)
                for j in range(2):
                    p = 2 * blk + j
                    ych = y2_v[:, p, eh]
                    dst = yo2[:, 512 * j:512 * j + 512]
                    if unit % 2 == 0:
                        nc.scalar.activation(dst, ych, AF.Identity,
                                             bias=shift[:, eh:eh + 1],
                                             scale=scl[:, eh:eh + 1])
                    else:
                        nc.vector.tensor_scalar(
                            dst, ych, scl[:, eh:eh + 1], shift[:, eh:eh + 1],
                            op0=AluOpType.mult, op1=AluOpType.add)
                    unit += 1
                eng = nc.sync if blk % 2 == 0 else nc.gpsimd
                eng.dma_start(out=out_r[eh, :, 1024 * blk:1024 * blk + 1024],
                              in_=yo2[:])

    nc.finalize()
    return nc


def _get_program():
    if "nc" not in _CACHE:
        _CACHE["nc"] = _build_program()
    return _CACHE["nc"]


def _make_in_maps(x, Wq_h, Wkv_h, Wout_h, bout_h, Wq_w, Wkv_w, Wout_w, bout_w,
                  conv_w, gamma, beta):
    import ml_dtypes
    f = np.float32
    bf = ml_dtypes.bfloat16

    shared = {
        "wq_w": np.ascontiguousarray(np.asarray(Wq_w, f).astype(bf)),
        "wk_w": np.ascontiguousarray(np.asarray(Wkv_w, f)[:, :C].astype(bf)),
        "wq_h": np.ascontiguousarray(np.asarray(Wq_h, f).astype(bf)),
        "wk_h": np.ascontiguousarray(np.asarray(Wkv_h, f)[:, :C].astype(bf)),
        "wv_w": np.ascontiguousarray(np.asarray(Wkv_w, f)[:, C:].astype(bf)),
        "wo_w": np.ascontiguousarray(np.asarray(Wout_w, f).astype(bf)),
        "wv_h": np.ascontiguousarray(np.asarray(Wkv_h, f)[:, C:].astype(bf)),
        "wo_h": np.ascontiguousarray(np.asarray(Wout_h, f).astype(bf)),
        "bsum": np.ascontiguousarray((np.asarray(bout_h, f) + np.asarray(bout_w, f)).reshape(C, 1)),
        "convA": np.ascontiguousarray(np.asarray(conv_w, f)[:C, :].astype(bf)),
        "convX": np.ascontiguousarray(np.asarray(conv_w, f)[C:, :].astype(bf)),
        "gamma2": np.ascontiguousarray(np.asarray(gamma, f).reshape(2, C).T),
        "beta2": np.ascontiguousarray(np.asarray(beta, f).reshape(2, C).T),
    }
    xb = np.asarray(x, f).astype(bf)
    return [{**shared, "x": np.ascontiguousarray(xb[b])} for b in range(B)]


def run(trace=False, **inputs):
    from concourse.bass_utils import run_bass_kernel_spmd

    nc = _get_program()
    in_maps = _make_in_maps(**inputs)
    res = run_bass_kernel_spmd(nc, in_maps, list(range(NCORES)), trace=trace)
    out = np.stack([res.results[b]["out"] for b in range(B)], axis=0)
    return out, res


def kernel(**inputs):
    out, _ = run(trace=False, **inputs)
    return out
